# revision 31
# baseline (speedup 1.0000x reference)
"""CharacterAwareEncoder kernel for Trainium2 (8 NeuronCores, data-parallel).

reference:
    word_embeds  = word_emb_table[word_ids]                  # [B, S, 412] gather
    char_features = sin(freqs * word_ids), 0 where id == 0   # [B, S, 100]
    out = concat([word_embeds, char_features], -1)           # [B, S, 512]

Sharding: word_ids [16, 2048] flattened to 32768 tokens, 4096 per core;
embedding table replicated (padded to 448 f32/row on host so each gathered
row is one 1792-B descriptor, the smallest 256-B-aligned row covering 412).

Per core, four independent lanes (CoreSim charges each DMA's transfer as an
exclusive hold on its issuing engine; transfers on different engines overlap
freely):
  Pool  - the SWDGE gather stream: 10 InstDMAGatherAnt chunks cover all 4096
          rows at ~2.92 ns/token into one [128, 32, 448] SBUF arena, plus the
          idx load and the tail embed stores.
  DVE   - sin range reduction: x = tok*freq (freqs broadcast from a single
          [128, 100] column block); y = x*INV2PI + 1.5*2^23 (magic
          round-to-nearest-even, replaces int casts and the range wrap);
          kf = y - MAGIC; r = Cody-Waite cascade; clamp to +-PI_SAFE.
          Blocks descend [10, 10, 10, 2] tiles so the last sin + sin store
          chain off the DVE tail is short.
  ACT   - the Sin activations plus mid-kernel embed stores; its queue after
          the last Sin holds only the tiny last sin store.
  SP    - consts load and the bulk of the embed stores.

The output is produced as two DRAM tensors - out_emb [4096, 412] (gather
layout, token = j*128 + p) and out_sin [4096, 100] (p-major layout,
token = p*32 + j, so each partition stores contiguous multi-KB runs) -
and the host concatenates columns during unsharding.

sin accuracy: x = freq*tok <= 3168 rad. k = RNE(x/2pi) exactly via magic
add/sub, r = ((x - k*c1) - k*c2) - k*c3 (Cody-Waite), clamp to +-PI_SAFE
so the ACT Sin table never sees |x| > pi. Worst case ~2e-4 abs err on the
~6e-5 fraction of elements within float rounding of an odd multiple of pi
(k off by one, |r| marginally > pi, clamped); everywhere else ~4e-7.
"""

import numpy as np

import concourse.bacc as bacc
import concourse.bass as bass
import concourse.mybir as mybir
import concourse.tile as tile
from concourse.bass_utils import run_bass_kernel_spmd

B, S = 16, 2048
V, D, H = 32000, 412, 100
OUT_D = 512
GW = 448                           # gathered row width (256-B aligned >= 412)
N_CORES = 8
P = 128
T_CORE = B * S // N_CORES          # 4096 tokens per core
N_TILES = T_CORE // P              # 32 tiles of 128 tokens
JPP = T_CORE // P                  # sin tokens per partition (p-major), 32

# gather chunks in tiles (sum = 32): small head chunk starts the store lanes
# early, small tail chunks keep the final chain short.
CHUNK_TILES = [2, 4, 4, 4, 4, 4, 4, 3, 2, 1]
# gather stream order: ACT's store chunks (3: tiles 10-14, 5: tiles 18-22)
# delivered early to fill ACT's pre-sin idle window.
CHUNK_ORDER = list(range(len(CHUNK_TILES)))
# sin-pipeline blocks in tiles: descending so the last blocks' range
# reduction, Sin, and sin store are all tiny.
BLK_TILES = [10, 10, 6, 4, 2]
WMAX = 16 * H

# schedule: interleaved emission program. Entries:
#   ("g", chunk_idx)            gather on Pool
#   ("blk", b)                  DVE range-reduction block b
#   ("sin", b)                  ACT Sin for block b
#   ("e", eng, t0, t1)          embed store tiles [t0, t1)
#   ("s", eng, b)               sin store for block b
# Best-found schedule (CoreSim 19600 ns/core vs 32872 baseline): kf of
# block 0 rides the ACT Copy activation (fills ACT's pre-sin idle,
# shortens the DVE chain); sin stores alternate SP/Pool; mid/late embed
# stores are <=2-tile pieces so no lane grabs a multi-us job right before
# a tail-critical Sin or sin store becomes ready; the last three tiles'
# stores spread across SP/Pool/Pool so the three lane tails overlap.
SCHEDULE = (
    [("blk", 0, True), ("sin", 0), ("blk", 1), ("sin", 1), ("blk", 2),
     ("sin", 2), ("blk", 3), ("sin", 3), ("blk", 4), ("sin", 4),
     ("s", "sp", 0), ("s", "pool", 1), ("s", "sp", 2), ("s", "pool", 3),
     ("s", "sp", 4),
     ("e", "sp", 0, 2), ("e", "sp", 2, 6), ("e", "sp", 6, 10),
     ("e", "act", 10, 12), ("e", "act", 12, 14), ("e", "sp", 14, 16),
     ("e", "sp", 16, 18), ("e", "act", 18, 20), ("e", "act", 20, 22),
     ("e", "sp", 22, 24), ("e", "sp", 24, 26), ("e", "act", 26, 28),
     ("e", "pool", 28, 29), ("e", "sp", 29, 30), ("e", "pool", 30, 31),
     ("e", "pool", 31, 32)]
)

_f32 = mybir.dt.float32
_i16 = mybir.dt.int16

_TWO_PI = 2.0 * np.pi
def _split_high(v):
    f = np.float32(v)
    return (f.view(np.uint32) & np.uint32(0xFFFFF000)).view(np.float32)
C1 = float(_split_high(_TWO_PI))
C2 = float(_split_high(_TWO_PI - C1))
C3 = float(np.float32(_TWO_PI - C1 - C2))
INV2PI = float(np.float32(1.0 / _TWO_PI))
MAGIC = float(np.float32(1.5 * 2.0**23))  # RNE quantizer for |y| < 2^22
PI_SAFE = float(np.nextafter(np.float32(np.pi), np.float32(0)))

GATHER_MODE = "v5"
_NC = {}


def _build_nc(mode=None, chunk_tiles=None, blk_tiles=None, schedule=None,
              consts_on_pool=False, chunk_order=None):
    chunk_tiles = chunk_tiles or CHUNK_TILES
    blk_tiles = blk_tiles or BLK_TILES
    schedule = schedule or SCHEDULE
    chunk_order = chunk_order or CHUNK_ORDER
    blk_start = np.cumsum([0] + list(blk_tiles))
    nc = bacc.Bacc("TRN2", target_bir_lowering=False, num_swdge_queues=1)
    # consts: [0:H] freqs, [H:H+JPP] p-major token ids as f32
    consts_t = nc.dram_tensor("consts", [P, H + JPP], _f32,
                              kind="ExternalInput")
    idx_t = nc.dram_tensor("idx16", [P, T_CORE // 16], _i16,
                           kind="ExternalInput")
    table_t = nc.dram_tensor("table", [V, GW], _f32, kind="ExternalInput")
    oemb_t = nc.dram_tensor("out_emb", [T_CORE, D], _f32,
                            kind="ExternalOutput")
    osin_t = nc.dram_tensor("out_sin", [T_CORE, H], _f32,
                            kind="ExternalOutput")

    chunk_start = np.cumsum([0] + list(chunk_tiles))

    with tile.TileContext(nc) as tc:
        with (
            tc.tile_pool(name="const", bufs=1) as cpool,
            tc.tile_pool(name="arena", bufs=1) as apool,
            tc.tile_pool(name="work", bufs=2) as wpool,
        ):
            idx_sb = cpool.tile([P, T_CORE // 16], _i16)
            consts_sb = cpool.tile([P, H + JPP], _f32)
            if consts_on_pool:
                # consts first on Pool: the sin pipeline's gate loads before
                # idx; same-engine SWDGE ordering lets the gathers follow
                # the idx write without a semaphore round-trip.
                nc.gpsimd.dma_start(out=consts_sb[:], in_=consts_t[:])
                nc.gpsimd.dma_start(out=idx_sb[:], in_=idx_t[:])
            else:
                # idx via Pool SWDGE: tiny engine hold, and the gather
                # stream engine owns its own critical input.
                nc.gpsimd.dma_start(out=idx_sb[:], in_=idx_t[:])
                nc.sync.dma_start(out=consts_sb[:], in_=consts_t[:])
            freqs_sb = consts_sb[:, 0:H]
            tokf = consts_sb[:, H : H + JPP]

            ch = apool.tile([P, N_TILES, GW], _f32)    # gathered rows
            r3 = apool.tile([P, JPP, H], _f32)         # clamped angles
            sinout = apool.tile([P, JPP, H], _f32)     # sin values (p-major)
            scratch = apool.tile([P, 1], _f32)

            # ACT warmup: force the Sin act-table load during the idle head.
            nc.vector.memset(scratch[:], 0.0)
            nc.scalar.activation(out=scratch[:], in_=scratch[:],
                                 func=mybir.ActivationFunctionType.Sin)

            def emit_gather(c):
                t0, t1 = chunk_start[c], chunk_start[c + 1]
                toks = (t1 - t0) * P
                nc.gpsimd.dma_gather(
                    ch[:, t0:t1, :],
                    table_t[:],
                    idx_sb[:, t0 * (P // 16) : t1 * (P // 16)],
                    toks, toks, GW,
                )

            def emit_block(b, kf_on_act=False):
                """DVE: x, y, kf, Cody-Waite, clamp for block b.
                kf_on_act routes the kf subtraction through the ACT Copy
                activation (fills ACT's pre-sin idle, shortens DVE)."""
                j0, j1 = blk_start[b], blk_start[b + 1]
                nt = j1 - j0
                w = nt * H
                tb = tokf[:, j0:j1]
                x = wpool.tile([P, WMAX], _f32, tag="x")
                nc.vector.tensor_tensor(
                    out=x[:, 0:w].rearrange("p (j h) -> p j h", j=nt),
                    in0=tb.to_broadcast([P, nt, H]),
                    in1=freqs_sb.rearrange("p (j h) -> p j h", j=1)
                    .to_broadcast([P, nt, H]),
                    op=mybir.AluOpType.mult,
                )
                y = wpool.tile([P, WMAX], _f32, tag="y")
                nc.vector.tensor_scalar(
                    out=y[:, 0:w], in0=x[:, 0:w], scalar1=INV2PI,
                    scalar2=MAGIC,
                    op0=mybir.AluOpType.mult, op1=mybir.AluOpType.add,
                )
                kf = wpool.tile([P, WMAX], _f32, tag="kf")
                if kf_on_act:
                    nc.scalar.activation(
                        out=kf[:, 0:w], in_=y[:, 0:w],
                        func=mybir.ActivationFunctionType.Copy,
                        scale=1.0, bias=-MAGIC,
                    )
                else:
                    nc.vector.tensor_scalar(
                        out=kf[:, 0:w], in0=y[:, 0:w], scalar1=-MAGIC,
                        scalar2=None, op0=mybir.AluOpType.add,
                    )
                r = wpool.tile([P, WMAX], _f32, tag="r")
                nc.vector.cody_waite_cascade(
                    out=r[:, 0:w], x=x[:, 0:w], k=kf[:, 0:w],
                    c1=C1, c2=C2, c3=C3,
                )
                nc.vector.tensor_scalar(
                    out=r3[:, j0:j1, :],
                    in0=r[:, 0:w].rearrange("p (j h) -> p j h", j=nt),
                    scalar1=PI_SAFE, scalar2=-PI_SAFE,
                    op0=mybir.AluOpType.min, op1=mybir.AluOpType.max,
                )

            def emit_sin(b):
                j0, j1 = blk_start[b], blk_start[b + 1]
                nc.scalar.activation(
                    out=sinout[:, j0:j1, :],
                    in_=r3[:, j0:j1, :],
                    func=mybir.ActivationFunctionType.Sin,
                )

            ENG = {"sp": nc.sync, "act": nc.scalar, "pool": nc.gpsimd}

            def emit_sstore(eng, b, j1=None):
                # p-major: one contiguous multi-KB run per partition.
                # (eng, b) stores sin block b; (eng, j0, j1) stores an
                # explicit tile range.
                if j1 is None:
                    j0, j1 = blk_start[b], blk_start[b + 1]
                else:
                    j0 = b
                ENG[eng].dma_start(
                    out=osin_t[:].rearrange("(p j) c -> p j c", p=P)
                    [:, j0:j1, :],
                    in_=sinout[:, j0:j1, :],
                )

            def emit_estore(eng, t0, t1):
                ENG[eng].dma_start(
                    out=oemb_t[t0 * P : t1 * P, :]
                    .rearrange("(j p) c -> p j c", p=P),
                    in_=ch[:, t0:t1, 0:D],
                )

            # ---- emission (priority = emission order for the tile
            # scheduler; lane = issuing engine) ----
            for c in chunk_order:
                emit_gather(c)                   # Pool stream
            for step in schedule:
                if step[0] == "blk":
                    emit_block(step[1], *step[2:])
                elif step[0] == "sin":
                    emit_sin(step[1])
                elif step[0] == "e":
                    emit_estore(step[1], step[2], step[3])
                elif step[0] == "s":
                    emit_sstore(step[1], *step[2:])
                else:
                    raise ValueError(step)
    nc.compile()
    return nc


def _get_nc(mode=None):
    if "v5" not in _NC:
        _NC["v5"] = _build_nc()
    return _NC["v5"]


def make_in_maps(word_ids, word_emb_table, mode=None):
    ids = np.ascontiguousarray(np.asarray(word_ids)).astype(np.int32).reshape(-1)
    table = np.asarray(word_emb_table, dtype=np.float32)
    padded = np.zeros((V, GW), np.float32)
    padded[:, 0:D] = table
    freqs_row = np.arange(H, dtype=np.float32) / np.float32(1000.0)

    in_maps = []
    for c in range(N_CORES):
        shard = ids[c * T_CORE : (c + 1) * T_CORE]
        consts = np.empty((P, H + JPP), np.float32)
        consts[:, 0:H] = freqs_row
        # p-major token layout for the sin pipeline: tok (p, j) = shard[p*JPP+j]
        consts[:, H:] = shard.reshape(P, JPP).astype(np.float32)
        # wrapped int16 layout for dma_gather: token i at [i % 16, i // 16],
        # replicated over the 8 groups of 16 partitions (one per Q7 core)
        base = shard.astype(np.int16).reshape(T_CORE // 16, 16).T  # [16, n/16]
        in_maps.append({
            "consts": consts,
            "table": padded,
            "idx16": np.ascontiguousarray(np.tile(base, (8, 1))),
        })
    return in_maps


def kernel(word_ids, word_emb_table):
    nc = _get_nc()
    in_maps = make_in_maps(word_ids, word_emb_table)
    res = run_bass_kernel_spmd(nc, in_maps, core_ids=list(range(N_CORES)))
    outs = []
    for r in res.results:
        emb = r["out_emb"]                       # [T_CORE, 412], token-major
        sin = r["out_sin"]                       # [T_CORE, 100], token-major
        outs.append(np.concatenate([emb, sin], axis=1))
    return np.concatenate(outs, axis=0).reshape(B, S, OUT_D)


# revision 34
# speedup vs baseline: 1.0310x; 1.0310x over previous
"""CharacterAwareEncoder kernel for Trainium2 (8 NeuronCores, data-parallel).

reference:
    word_embeds  = word_emb_table[word_ids]                  # [B, S, 412] gather
    char_features = sin(freqs * word_ids), 0 where id == 0   # [B, S, 100]
    out = concat([word_embeds, char_features], -1)           # [B, S, 512]

Sharding: word_ids [16, 2048] flattened to 32768 tokens, 4096 per core;
embedding table replicated (padded to 448 f32/row on host so each gathered
row is one 1792-B descriptor, the smallest 256-B-aligned row covering 412).

Per core, four independent lanes (CoreSim charges each DMA's transfer as an
exclusive hold on its issuing engine; transfers on different engines overlap
freely):
  Pool  - the SWDGE gather stream: 10 InstDMAGatherAnt chunks cover all 4096
          rows at ~2.92 ns/token into one [128, 32, 448] SBUF arena, plus the
          idx load and the tail embed stores.
  DVE   - sin range reduction: x = tok*freq (freqs broadcast from a single
          [128, 100] column block); y = x*INV2PI + 1.5*2^23 (magic
          round-to-nearest-even, replaces int casts and the range wrap);
          kf = y - MAGIC; r = Cody-Waite cascade; clamp to +-PI_SAFE.
          Blocks descend [10, 10, 10, 2] tiles so the last sin + sin store
          chain off the DVE tail is short.
  ACT   - the Sin activations plus mid-kernel embed stores; its queue after
          the last Sin holds only the tiny last sin store.
  SP    - consts load and the bulk of the embed stores.

The output is produced as two DRAM tensors - out_emb [4096, 412] (gather
layout, token = j*128 + p) and out_sin [4096, 100] (p-major layout,
token = p*32 + j, so each partition stores contiguous multi-KB runs) -
and the host concatenates columns during unsharding.

sin accuracy: x = freq*tok <= 3168 rad. k = RNE(x/2pi) exactly via magic
add/sub, r = ((x - k*c1) - k*c2) - k*c3 (Cody-Waite), clamp to +-PI_SAFE
so the ACT Sin table never sees |x| > pi. Worst case ~2e-4 abs err on the
~6e-5 fraction of elements within float rounding of an odd multiple of pi
(k off by one, |r| marginally > pi, clamped); everywhere else ~4e-7.
"""

import numpy as np

import concourse.bacc as bacc
import concourse.bass as bass
import concourse.mybir as mybir
import concourse.tile as tile
from concourse.bass_utils import run_bass_kernel_spmd

B, S = 16, 2048
V, D, H = 32000, 412, 100
OUT_D = 512
GW = 448                           # gathered row width (256-B aligned >= 412)
N_CORES = 8
P = 128
T_CORE = B * S // N_CORES          # 4096 tokens per core
N_TILES = T_CORE // P              # 32 tiles of 128 tokens
JPP = T_CORE // P                  # sin tokens per partition (p-major), 32

# gather chunks in tiles (sum = 32): small head chunk starts the store lanes
# early, small tail chunks keep the final chain short.
CHUNK_TILES = [2, 4, 4, 4, 4, 4, 4, 3, 2, 1]
# gather stream order: ACT's store chunks (3: tiles 10-14, 5: tiles 18-22)
# delivered early to fill ACT's pre-sin idle window.
CHUNK_ORDER = list(range(len(CHUNK_TILES)))
# sin-pipeline blocks in tiles: descending so the last blocks' range
# reduction, Sin, and sin store are all tiny.
BLK_TILES = [10, 10, 6, 4, 2]
WMAX = 16 * H

# schedule: interleaved emission program. Entries:
#   ("g", chunk_idx)            gather on Pool
#   ("blk", b)                  DVE range-reduction block b
#   ("sin", b)                  ACT Sin for block b
#   ("e", eng, t0, t1)          embed store tiles [t0, t1)
#   ("s", eng, b)               sin store for block b
# Best-found schedule (CoreSim 19600 ns/core vs 32872 baseline): kf of
# block 0 rides the ACT Copy activation (fills ACT's pre-sin idle,
# shortens the DVE chain); sin stores alternate SP/Pool; mid/late embed
# stores are <=2-tile pieces so no lane grabs a multi-us job right before
# a tail-critical Sin or sin store becomes ready; the last three tiles'
# stores spread across SP/Pool/Pool so the three lane tails overlap.
SCHEDULE = (
    [("blk", 0, True), ("sin", 0), ("blk", 1), ("sin", 1), ("blk", 2),
     ("sin", 2), ("blk", 3), ("sin", 3), ("blk", 4), ("sin", 4),
     ("s", "sp", 0), ("s", "pool", 1), ("s", "sp", 2), ("s", "sp", 3),
     ("s", "sp", 4),
     ("e", "sp", 0, 2), ("e", "sp", 2, 6), ("e", "sp", 6, 10),
     ("e", "act", 10, 12), ("e", "act", 12, 14), ("e", "sp", 14, 16),
     ("e", "sp", 16, 18), ("e", "act", 18, 20), ("e", "act", 20, 22),
     ("e", "sp", 22, 24), ("e", "sp", 24, 26), ("e", "act", 26, 28),
     ("e", "pool", 28, 29), ("e", "pool", 29, 30), ("e", "pool", 30, 31),
     ("e", "pool", 31, 32)]
)

_f32 = mybir.dt.float32
_i16 = mybir.dt.int16

_TWO_PI = 2.0 * np.pi
def _split_high(v):
    f = np.float32(v)
    return (f.view(np.uint32) & np.uint32(0xFFFFF000)).view(np.float32)
C1 = float(_split_high(_TWO_PI))
C2 = float(_split_high(_TWO_PI - C1))
C3 = float(np.float32(_TWO_PI - C1 - C2))
INV2PI = float(np.float32(1.0 / _TWO_PI))
MAGIC = float(np.float32(1.5 * 2.0**23))  # RNE quantizer for |y| < 2^22
PI_SAFE = float(np.nextafter(np.float32(np.pi), np.float32(0)))

GATHER_MODE = "v5"
_NC = {}


def _build_nc(mode=None, chunk_tiles=None, blk_tiles=None, schedule=None,
              consts_on_pool=False, chunk_order=None, idx_split=0):
    chunk_tiles = chunk_tiles or CHUNK_TILES
    blk_tiles = blk_tiles or BLK_TILES
    schedule = schedule or SCHEDULE
    chunk_order = chunk_order or CHUNK_ORDER
    blk_start = np.cumsum([0] + list(blk_tiles))
    nc = bacc.Bacc("TRN2", target_bir_lowering=False, num_swdge_queues=1)
    # consts: [0:H] freqs, [H:H+JPP] p-major token ids as f32
    consts_t = nc.dram_tensor("consts", [P, H + JPP], _f32,
                              kind="ExternalInput")
    idx_t = nc.dram_tensor("idx16", [P, T_CORE // 16], _i16,
                           kind="ExternalInput")
    table_t = nc.dram_tensor("table", [V, GW], _f32, kind="ExternalInput")
    oemb_t = nc.dram_tensor("out_emb", [T_CORE, D], _f32,
                            kind="ExternalOutput")
    osin_t = nc.dram_tensor("out_sin", [T_CORE, H], _f32,
                            kind="ExternalOutput")

    chunk_start = np.cumsum([0] + list(chunk_tiles))

    with tile.TileContext(nc) as tc:
        with (
            tc.tile_pool(name="const", bufs=1) as cpool,
            tc.tile_pool(name="arena", bufs=1) as apool,
            tc.tile_pool(name="work", bufs=2) as wpool,
        ):
            idx_sb = cpool.tile([P, T_CORE // 16], _i16)
            consts_sb = cpool.tile([P, H + JPP], _f32)
            if consts_on_pool:
                # consts first on Pool: the sin pipeline's gate loads before
                # idx; same-engine SWDGE ordering lets the gathers follow
                # the idx write without a semaphore round-trip.
                nc.gpsimd.dma_start(out=consts_sb[:], in_=consts_t[:])
                nc.gpsimd.dma_start(out=idx_sb[:], in_=idx_t[:])
            elif idx_split:
                # Split the idx load: a small head slice on Pool (its gather
                # follows by same-engine ordering with no semaphore
                # round-trip, starting the stream ~0.3us earlier); the rest
                # rides ACT's idle head.
                nc.gpsimd.dma_start(out=idx_sb[:, 0:idx_split],
                                    in_=idx_t[:, 0:idx_split])
                nc.scalar.dma_start(out=idx_sb[:, idx_split:],
                                    in_=idx_t[:, idx_split:])
                nc.sync.dma_start(out=consts_sb[:], in_=consts_t[:])
            else:
                # idx via Pool SWDGE: tiny engine hold, and the gather
                # stream engine owns its own critical input.
                nc.gpsimd.dma_start(out=idx_sb[:], in_=idx_t[:])
                nc.sync.dma_start(out=consts_sb[:], in_=consts_t[:])
            freqs_sb = consts_sb[:, 0:H]
            tokf = consts_sb[:, H : H + JPP]

            ch = apool.tile([P, N_TILES, GW], _f32)    # gathered rows
            r3 = apool.tile([P, JPP, H], _f32)         # clamped angles
            sinout = apool.tile([P, JPP, H], _f32)     # sin values (p-major)
            scratch = apool.tile([P, 1], _f32)

            # ACT warmup: force the Sin act-table load during the idle head.
            nc.vector.memset(scratch[:], 0.0)
            nc.scalar.activation(out=scratch[:], in_=scratch[:],
                                 func=mybir.ActivationFunctionType.Sin)

            def emit_gather(c):
                t0, t1 = chunk_start[c], chunk_start[c + 1]
                toks = (t1 - t0) * P
                nc.gpsimd.dma_gather(
                    ch[:, t0:t1, :],
                    table_t[:],
                    idx_sb[:, t0 * (P // 16) : t1 * (P // 16)],
                    toks, toks, GW,
                )

            def emit_block(b, kf_on_act=False):
                """DVE: x, y, kf, Cody-Waite, clamp for block b.
                kf_on_act routes the kf subtraction through the ACT Copy
                activation (fills ACT's pre-sin idle, shortens DVE)."""
                j0, j1 = blk_start[b], blk_start[b + 1]
                nt = j1 - j0
                w = nt * H
                tb = tokf[:, j0:j1]
                x = wpool.tile([P, WMAX], _f32, tag="x")
                nc.vector.tensor_tensor(
                    out=x[:, 0:w].rearrange("p (j h) -> p j h", j=nt),
                    in0=tb.to_broadcast([P, nt, H]),
                    in1=freqs_sb.rearrange("p (j h) -> p j h", j=1)
                    .to_broadcast([P, nt, H]),
                    op=mybir.AluOpType.mult,
                )
                y = wpool.tile([P, WMAX], _f32, tag="y")
                nc.vector.tensor_scalar(
                    out=y[:, 0:w], in0=x[:, 0:w], scalar1=INV2PI,
                    scalar2=MAGIC,
                    op0=mybir.AluOpType.mult, op1=mybir.AluOpType.add,
                )
                kf = wpool.tile([P, WMAX], _f32, tag="kf")
                if kf_on_act:
                    nc.scalar.activation(
                        out=kf[:, 0:w], in_=y[:, 0:w],
                        func=mybir.ActivationFunctionType.Copy,
                        scale=1.0, bias=-MAGIC,
                    )
                else:
                    nc.vector.tensor_scalar(
                        out=kf[:, 0:w], in0=y[:, 0:w], scalar1=-MAGIC,
                        scalar2=None, op0=mybir.AluOpType.add,
                    )
                r = wpool.tile([P, WMAX], _f32, tag="r")
                nc.vector.cody_waite_cascade(
                    out=r[:, 0:w], x=x[:, 0:w], k=kf[:, 0:w],
                    c1=C1, c2=C2, c3=C3,
                )
                nc.vector.tensor_scalar(
                    out=r3[:, j0:j1, :],
                    in0=r[:, 0:w].rearrange("p (j h) -> p j h", j=nt),
                    scalar1=PI_SAFE, scalar2=-PI_SAFE,
                    op0=mybir.AluOpType.min, op1=mybir.AluOpType.max,
                )

            def emit_sin(b):
                j0, j1 = blk_start[b], blk_start[b + 1]
                nc.scalar.activation(
                    out=sinout[:, j0:j1, :],
                    in_=r3[:, j0:j1, :],
                    func=mybir.ActivationFunctionType.Sin,
                )

            ENG = {"sp": nc.sync, "act": nc.scalar, "pool": nc.gpsimd}

            def emit_sstore(eng, b, j1=None):
                # p-major: one contiguous multi-KB run per partition.
                # (eng, b) stores sin block b; (eng, j0, j1) stores an
                # explicit tile range.
                if j1 is None:
                    j0, j1 = blk_start[b], blk_start[b + 1]
                else:
                    j0 = b
                ENG[eng].dma_start(
                    out=osin_t[:].rearrange("(p j) c -> p j c", p=P)
                    [:, j0:j1, :],
                    in_=sinout[:, j0:j1, :],
                )

            def emit_estore(eng, t0, t1):
                ENG[eng].dma_start(
                    out=oemb_t[t0 * P : t1 * P, :]
                    .rearrange("(j p) c -> p j c", p=P),
                    in_=ch[:, t0:t1, 0:D],
                )

            # ---- emission (priority = emission order for the tile
            # scheduler; lane = issuing engine) ----
            for c in chunk_order:
                emit_gather(c)                   # Pool stream
            for step in schedule:
                if step[0] == "blk":
                    emit_block(step[1], *step[2:])
                elif step[0] == "sin":
                    emit_sin(step[1])
                elif step[0] == "e":
                    emit_estore(step[1], step[2], step[3])
                elif step[0] == "s":
                    emit_sstore(step[1], *step[2:])
                else:
                    raise ValueError(step)
    nc.compile()
    return nc


def _get_nc(mode=None):
    if "v5" not in _NC:
        _NC["v5"] = _build_nc()
    return _NC["v5"]


def make_in_maps(word_ids, word_emb_table, mode=None):
    ids = np.ascontiguousarray(np.asarray(word_ids)).astype(np.int32).reshape(-1)
    table = np.asarray(word_emb_table, dtype=np.float32)
    padded = np.zeros((V, GW), np.float32)
    padded[:, 0:D] = table
    freqs_row = np.arange(H, dtype=np.float32) / np.float32(1000.0)

    in_maps = []
    for c in range(N_CORES):
        shard = ids[c * T_CORE : (c + 1) * T_CORE]
        consts = np.empty((P, H + JPP), np.float32)
        consts[:, 0:H] = freqs_row
        # p-major token layout for the sin pipeline: tok (p, j) = shard[p*JPP+j]
        consts[:, H:] = shard.reshape(P, JPP).astype(np.float32)
        # wrapped int16 layout for dma_gather: token i at [i % 16, i // 16],
        # replicated over the 8 groups of 16 partitions (one per Q7 core)
        base = shard.astype(np.int16).reshape(T_CORE // 16, 16).T  # [16, n/16]
        in_maps.append({
            "consts": consts,
            "table": padded,
            "idx16": np.ascontiguousarray(np.tile(base, (8, 1))),
        })
    return in_maps


def kernel(word_ids, word_emb_table):
    nc = _get_nc()
    in_maps = make_in_maps(word_ids, word_emb_table)
    res = run_bass_kernel_spmd(nc, in_maps, core_ids=list(range(N_CORES)))
    outs = []
    for r in res.results:
        emb = r["out_emb"]                       # [T_CORE, 412], token-major
        sin = r["out_sin"]                       # [T_CORE, 100], token-major
        outs.append(np.concatenate([emb, sin], axis=1))
    return np.concatenate(outs, axis=0).reshape(B, S, OUT_D)
